# revision 62
# baseline (speedup 1.0000x reference)
"""Causal self-attention kernel for Trainium2 (8 NeuronCores, Bass/Tile).

Problem: B=4, S=2048, D=1024, H=16, HD=64, fp32.
Sharding: core c -> (batch b = c//2, head-group hg = c%2). Each core computes
attention for its batch over 8 heads (features hg*512..hg*512+511 of each of
the k/q/v projection chunks), plus the partial output projection
attn_out_slice @ W_out[rows of this head group].  Host sums the two partial
out-projections per batch and adds nothing else (b_out folded in on hg==0).

Device-side layout choices (no on-device transposes anywhere):
  - host provides x^T [D, S]; K^T/Q^T are produced feature-major [F, S] in
    bf16 by using W as the matmul stationary operand; V is produced
    seq-major [S, F] by using x^T as the stationary operand. All PSUM
    evictions (bias adds) run on the vector engine with stride-0
    broadcast-bias APs, keeping the scalar engine free for exp().
  - attention uses the scores-transposed layout S^T[k, q]: QK^T pairs of
    heads run row-tiled (head A in PE rows 0-63, head B in rows 64-127),
    exp() on the scalar engine (no max subtraction: scores ~ N(0,1)),
    causal masking as a 0/1 multiply on band tiles only.
  - AV-denominator fusion: V tiles carry a trailing ones column per head
    ([128, 8, 65]); each head's AV matmul uses a [128, 65] stationary so
    partition 64 of its PSUM bank accumulates the softmax denominator for
    free (no separate ones-matmuls, no separate denominator bank). The
    reciprocals are broadcast across partitions via a DRAM round-trip DMA
    and the normalization multiplies are deferred one q-block so the
    round-trip latency hides behind compute.
  - weights stream in batched DMAs (one 256KB transfer per K/Q block per
    head-pair, one 1MB transfer for W_out); hp=0's K/Q weights are issued
    before the V matmuls so the K-projection starts as the V phase drains.

Timing: `time_kernel` measures the slope (T_reps - T_1)/(reps - 1) between
a single-shot NEFF and one with an on-device For_i rep loop, which cancels
the 30-80ms axon-tunnel dispatch round-trip exactly and yields the marginal
HW cost of one full kernel execution (HBM loads and stores included).
"""

import contextlib
import math
import os
from contextlib import ExitStack

import numpy as np
from ml_dtypes import bfloat16

import concourse.bass as bass
import concourse.tile as tile
from concourse import bacc, mybir
from concourse.bass_utils import run_bass_kernel_spmd

F32 = mybir.dt.float32
BF16 = mybir.dt.bfloat16

# Matmul compute dtype: float32r (TF32-like, 1 cycle/row at N>=256) unless
# overridden for an accuracy fallback.
_MM_DT = {
    "f32r": mybir.dt.float32r,
    "f32": mybir.dt.float32,
}[os.environ.get("KERNEL_MM_DT", "f32r")]


def _r(ap):
    """Reinterpret an fp32 AP as the matmul compute dtype (same bytes)."""
    if _MM_DT == mybir.dt.float32:
        return ap
    return ap.bitcast(_MM_DT)


def build_nc(S=2048, D=1024, H_pc=8, HD=64, NQ=512, KT=128, reps=1,
             interleave_outproj=False):
    """Build the single-core Bass program (identical program on all cores).

    reps > 1 wraps the whole body in an on-device For_i loop that re-executes
    the identical (idempotent) computation; used for steady-state timing.
    """
    F = H_pc * HD          # per-core feature width of each of k/q/v (512)
    HP = F // 128          # head-pairs == 128-wide feature tiles (4)
    DKT = D // 128         # contraction tiles over d_model (8)
    NSEQ = S // NQ         # q blocks (4)
    NST = S // KT          # seq tiles for V (16)
    DM = D // 128          # output d_model tiles (8)
    NCH = S // 512         # 512-wide seq chunks for projections (4)
    BAND = NQ // KT        # k-tiles per q-block on the causal diagonal (4)

    nc = bacc.Bacc("TRN2", target_bir_lowering=False, debug=False, num_devices=8)

    x_t = nc.dram_tensor("x_t", [D, S], BF16, kind="ExternalInput").ap()
    w_k = nc.dram_tensor("w_k", [D, F], BF16, kind="ExternalInput").ap()
    w_q = nc.dram_tensor("w_q", [D, F], BF16, kind="ExternalInput").ap()
    w_v = nc.dram_tensor("w_v", [D, F], BF16, kind="ExternalInput").ap()
    b_k = nc.dram_tensor("b_k", [F, 1], F32, kind="ExternalInput").ap()
    b_q = nc.dram_tensor("b_q", [F, 1], F32, kind="ExternalInput").ap()
    b_v = nc.dram_tensor("b_v", [F], F32, kind="ExternalInput").ap()
    w_o = nc.dram_tensor("w_o", [F, D], BF16, kind="ExternalInput").ap()
    b_o = nc.dram_tensor("b_o", [D, 1], F32, kind="ExternalInput").ap()
    masks = nc.dram_tensor("masks", [128, 2, 128], BF16, kind="ExternalInput").ap()
    out_t = nc.dram_tensor("out_t", [D, S], F32, kind="ExternalOutput").ap()

    scale = 1.0 / math.sqrt(HD)

    with tile.TileContext(nc) as tc, \
         (tc.For_i(0, reps, 1) if reps > 1 else contextlib.nullcontext()), \
         ExitStack() as ctx:
        consts = ctx.enter_context(tc.tile_pool(name="consts", bufs=1))
        # per-partition bias columns for the feature-major K/Q projections
        bk_sb = consts.tile([128, HP], F32, tag="bk")
        bq_sb = consts.tile([128, HP], F32, tag="bq")
        nc.sync.dma_start(out=bk_sb, in_=b_k.rearrange("(m p) one -> p (m one)", p=128))
        nc.sync.dma_start(out=bq_sb, in_=b_q.rearrange("(m p) one -> p (m one)", p=128))
        # V bias broadcast along partitions (bias varies along the free dim)
        bv_sb = consts.tile([128, F], F32, tag="bv")
        bv_bcast = bass.AP(tensor=b_v.tensor, offset=b_v.offset, ap=[[0, 128], [1, F]])
        nc.sync.dma_start(out=bv_sb, in_=bv_bcast)
        bo_sb = consts.tile([128, DM], F32, tag="bo")
        nc.sync.dma_start(out=bo_sb, in_=b_o.rearrange("(m p) one -> p (m one)", p=128))
        # persistent activations
        big = ctx.enter_context(tc.tile_pool(name="big", bufs=1))
        kT = [big.tile([128, S], BF16, tag=f"kT{m}", name=f"kT{m}") for m in range(HP)]
        qT = [big.tile([128, S], BF16, tag=f"qT{m}", name=f"qT{m}") for m in range(HP)]
        v = [big.tile([128, H_pc, HD + 1], BF16, tag=f"v{st}", name=f"v{st}")
             for st in range(NST)]
        aT = [big.tile([128, S], BF16, tag=f"aT{m}", name=f"aT{m}") for m in range(HP)]

        # ---- Phases A+B interleaved: V, then per head-pair {K,Q proj; attention} ----
        # All [128,512] PSUM accumulations (V-proj, K/Q-proj, scores) share one
        # 4-buffer pool so projection and attention pipelines coexist in the
        # 8 PSUM banks and the scheduler can overlap them across head-pairs.
        with tc.tile_pool(name="xp", bufs=1) as xp, \
             tc.tile_pool(name="mk", bufs=1) as mk, \
             tc.tile_pool(name="wsp", bufs=4) as wsp, \
             tc.tile_pool(name="pt_pool", bufs=6) as ptp, \
             tc.tile_pool(name="r2_pool", bufs=2) as r2p, \
             tc.tile_pool(name="sp", bufs=2, space="PSUM") as sp, \
             tc.tile_pool(name="op", bufs=2, space="PSUM") as op, \
             tc.tile_pool(name="dscr", bufs=4, space="DRAM") as dscr, \
             tc.tile_pool(name="wop", bufs=1) as wop, \
             tc.tile_pool(name="os", bufs=6) as osb:
            maskt = mk.tile([128, 2, 128], BF16, tag="mask", name="maskt")
            nc.sync.dma_start(out=maskt, in_=masks)
            xt = [xp.tile([128, S], BF16, tag=f"x{k}", name=f"x{k}") for k in range(DKT)]

            def load_kq_weights(hp):
                """One batched 256KB DMA per K/Q weight block for a head-pair.

                Layout: [128, DKT*128] where column block k holds
                W[k*128:(k+1)*128, hp*128:(hp+1)*128]."""
                wts = []
                for wdram in (w_k, w_q):
                    wt = wsp.tile([128, DKT, 128], BF16, tag="w", name="wt")
                    nc.sync.dma_start(
                        out=wt,
                        in_=wdram[:, hp * 128:(hp + 1) * 128].rearrange(
                            "(k p) f -> p k f", p=128),
                    )
                    wts.append(wt)
                return wts

            # V first (its weights can be released before the K/Q W stream);
            # interleave wv/x loads so the first accumulation starts early;
            # x rows in halves so early seq-tiles unblock sooner
            with tc.tile_pool(name="wvp", bufs=1) as wvp:
                wv = [wvp.tile([128, F], BF16, tag=f"wv{k}", name=f"wv{k}") for k in range(DKT)]
                for k in range(DKT):
                    nc.sync.dma_start(out=wv[k], in_=w_v[k * 128:(k + 1) * 128, :])
                    nc.sync.dma_start(out=xt[k][:, 0:S // 2],
                                      in_=x_t[k * 128:(k + 1) * 128, 0:S // 2])
                for k in range(DKT):
                    nc.sync.dma_start(out=xt[k][:, S // 2:S],
                                      in_=x_t[k * 128:(k + 1) * 128, S // 2:S])
                # hp=0's K/Q weights: in flight during the V matmuls so the
                # K-projection starts the moment the V phase drains
                wts0 = load_kq_weights(0)
                for st in range(NST):
                    ps = sp.tile([128, 2 * NQ], F32, tag="s")
                    for k in range(DKT):
                        nc.tensor.matmul(
                            ps[:, 0:F], xt[k][:, st * 128:(st + 1) * 128], wv[k],
                            start=(k == 0), stop=(k == DKT - 1),
                        )
                    nc.vector.tensor_add(
                        v[st][:, :, 0:HD],
                        ps[:, 0:F].rearrange("p (h f) -> p h f", h=H_pc),
                        bv_sb.rearrange("p (h f) -> p h f", h=H_pc),
                    )
                    # ones column: the AV matmul's 65th stationary column makes
                    # partition 64 of the output accumulate the softmax
                    # denominator for free
                    nc.vector.memset(v[st][:, :, HD:HD + 1], 1.0)

            cA = slice(0, 64)
            cB = slice(64, 128)

            # Output projection per 512-chunk through the shared score pool;
            # W_out for this core is 1 MB bf16: preload it in ONE DMA.
            # Layout: [128, HP*D]; column block k*D + mo*128 holds
            # W_out[k*128:(k+1)*128, mo*128:(mo+1)*128].
            wo = wop.tile([128, HP, D], BF16, tag="wo", name="wo")
            nc.sync.dma_start(out=wo, in_=w_o.rearrange("(k p) d -> p k d", p=128))

            def _bc(col, n):
                """[128,1] AP -> [128,n] stride-0 broadcast along the free dim."""
                return bass.AP(tensor=col.tensor, offset=col.offset,
                               ap=[list(col.ap[0]), [0, n]])

            def outproj_chunk(nch):
                for mo2 in range(DM // 2):
                    ps2 = sp.tile([128, 2 * NQ], F32, tag="s", name="ps_o2")
                    for half in range(2):
                        mo = 2 * mo2 + half
                        for k in range(HP):
                            nc.tensor.matmul(
                                ps2[:, half * 512:(half + 1) * 512],
                                wo[:, k, mo * 128:(mo + 1) * 128],
                                aT[k][:, nch * 512:(nch + 1) * 512],
                                start=(k == 0), stop=(k == HP - 1),
                            )
                    ot = osb.tile([128, 2, 512], F32, tag="ot")
                    bcol = bo_sb[:, 2 * mo2:2 * mo2 + 2]
                    b3 = bass.AP(tensor=bcol.tensor, offset=bcol.offset,
                                 ap=[list(bcol.ap[0]), list(bcol.ap[1]), [0, 512]])
                    nc.vector.tensor_add(
                        ot, ps2.rearrange("p (m q) -> p m q", m=2), b3)
                    for half in range(2):
                        mo = 2 * mo2 + half
                        nc.sync.dma_start(
                            out=out_t[mo * 128:(mo + 1) * 128,
                                      nch * 512:(nch + 1) * 512],
                            in_=ot[:, half, :],
                        )

            # deferred softmax tails: (hp, qi, o2, reA, reB) awaiting the
            # normalization multiply (the reciprocal -> DRAM -> broadcast
            # chain needs a couple of k-tile rounds of latency to hide)
            pending = []

            def flush_tail():
                while pending:
                    hp_, qi_, o2_, re2_ = pending.pop(0)
                    qs_ = slice(qi_ * NQ, (qi_ + 1) * NQ)
                    nc.vector.tensor_mul(
                        aT[hp_][0:64, qs_], o2_[0:64, 0, :], re2_[:, 0, :])
                    nc.vector.tensor_mul(
                        aT[hp_][64:128, qs_], o2_[0:64, 1, :], re2_[:, 1, :])
                    if interleave_outproj and hp_ == HP - 1:
                        # all head-pairs' aT for this chunk are done: overlap
                        # the output projection with the remaining q-blocks
                        outproj_chunk(qi_)

            for hp in range(HP):
                # K and Q projections for this head-pair's feature tile
                wts = wts0 if hp == 0 else load_kq_weights(hp)
                for (wt, bias_sb, dstT) in ((wts[0], bk_sb, kT), (wts[1], bq_sb, qT)):
                    # two 512-chunks per [128,1024] psum tile -> one paired
                    # eviction (halves the DVE op count and its drain cost)
                    for nch2 in range(NCH // 2):
                        ps2 = sp.tile([128, 2 * NQ], F32, tag="s", name="ps_kq")
                        for half in range(2):
                            nch = 2 * nch2 + half
                            for k in range(DKT):
                                nc.tensor.matmul(
                                    ps2[:, half * 512:(half + 1) * 512], wt[:, k],
                                    xt[k][:, nch * 512:(nch + 1) * 512],
                                    start=(k == 0), stop=(k == DKT - 1),
                                )
                        nc.vector.tensor_add(
                            dstT[hp][:, nch2 * 1024:(nch2 + 1) * 1024], ps2,
                            _bc(bias_sb[:, hp:hp + 1], 1024),
                        )

                # attention for this head-pair
                for qi in range(NSEQ):
                    nkt = (qi + 1) * BAND
                    # two PSUM banks: bank h holds head h's AV accumulation in
                    # partitions 0-63 and its softmax denominator in partition 64
                    o2 = op.tile([128, 2, NQ], F32, tag="o")
                    for kt in range(nkt):
                        ks = slice(kt * 128, (kt + 1) * 128)
                        j = kt - (nkt - BAND)
                        # valid q-subrange of this k-tile: q_local >= 128*j
                        lo = 128 * j if j > 0 else 0
                        s2 = sp.tile([128, 2 * NQ], F32, tag="s")
                        qsub = slice(qi * NQ + lo, (qi + 1) * NQ)
                        nc.tensor.matmul(
                            s2[:, lo:NQ], kT[hp][cA, ks], qT[hp][cA, qsub],
                            start=True, stop=True, tile_position=(0, 0),
                        )
                        nc.tensor.matmul(
                            s2[:, NQ + lo:2 * NQ], kT[hp][cB, ks],
                            qT[hp][cB, qsub],
                            start=True, stop=True, tile_position=(64, 0),
                        )
                        pt = ptp.tile([128, 2 * NQ], BF16, tag="p")
                        s2_3 = s2.rearrange("p (h q) -> p h q", h=2)
                        pt_3 = pt.rearrange("p (h q) -> p h q", h=2)
                        nc.scalar.activation(
                            pt_3[:, :, lo:NQ], s2_3[:, :, lo:NQ],
                            mybir.ActivationFunctionType.Exp, scale=scale,
                        )
                        if j >= 0:
                            # triangle mask on the first 128 valid columns
                            nc.vector.tensor_mul(
                                pt_3[:, :, lo:lo + 128], pt_3[:, :, lo:lo + 128],
                                maskt,
                            )
                        first, last = (kt == 0), (kt == nkt - 1)
                        nc.tensor.matmul(
                            o2[0:65, 0, lo:NQ], v[kt][:, 2 * hp, :],
                            pt[:, lo:NQ],
                            start=first, stop=last,
                            skip_group_check=True,
                        )
                        nc.tensor.matmul(
                            o2[0:65, 1, lo:NQ], v[kt][:, 2 * hp + 1, :],
                            pt[:, NQ + lo:2 * NQ],
                            start=first, stop=last,
                            skip_group_check=True,
                        )
                        if kt == 2:
                            # previous q-block's deferred normalization
                            flush_tail()
                    # reciprocals of the accumulated denominators (PSUM
                    # partition 64 of each bank), then broadcast across 64
                    # partitions via a DRAM round-trip (DMA replicates a DRAM
                    # row across partitions; engines cannot broadcast cheaply)
                    r2ab = r2p.tile([1, 2, NQ], F32, tag="r2ab")
                    nc.vector.reciprocal(r2ab, o2[64:65, :, :])
                    sab = dscr.tile([1, 2 * NQ], F32, tag="sab")
                    nc.sync.dma_start(out=sab, in_=r2ab)
                    re2 = r2p.tile([64, 2, NQ], F32, tag="re2")
                    nc.sync.dma_start(out=re2, in_=bass.AP(
                        tensor=sab.tensor, offset=sab.offset,
                        ap=[[0, 64], [1, 2 * NQ]]))
                    pending.append((hp, qi, o2, re2))

            flush_tail()
            if not interleave_outproj:
                for nch in range(NCH):
                    outproj_chunk(nch)


    nc.compile()
    return nc


def make_masks(NQ=512, KT=128):
    # triangle mask for the 128-wide causal boundary, duplicated for 2 heads
    k = np.arange(128)[:, None]
    c = np.arange(128)[None, :]
    keep = (c >= k).astype(np.float32)
    return np.stack([keep, keep], axis=1)  # [128, 2, 128]


def make_in_maps(x, W_in, b_in, W_out, b_out, S, D, H_pc, HD):
    """Build the 8 per-core input maps. Core c -> (batch c//2, head-group c%2)."""
    F = H_pc * HD
    B = x.shape[0]
    n_hg = D // F  # 2
    masks = make_masks()
    in_maps = []
    for c in range(B * n_hg):
        b, hg = c // n_hg, c % n_hg
        cols = slice(hg * F, (hg + 1) * F)
        # W_in chunk order (torch.chunk in the reference): k, q, v
        wk = np.ascontiguousarray(W_in[:, 0 * D:1 * D][:, cols])
        wq = np.ascontiguousarray(W_in[:, 1 * D:2 * D][:, cols])
        wv = np.ascontiguousarray(W_in[:, 2 * D:3 * D][:, cols])
        bk = np.ascontiguousarray(b_in[0 * D:1 * D][cols]).reshape(F, 1)
        bq = np.ascontiguousarray(b_in[1 * D:2 * D][cols]).reshape(F, 1)
        bv = np.ascontiguousarray(b_in[2 * D:3 * D][cols])
        wo = np.ascontiguousarray(W_out[cols, :])
        bo = (b_out if hg == 0 else np.zeros_like(b_out)).reshape(D, 1)
        in_maps.append({
            "x_t": np.ascontiguousarray(x[b].T).astype(bfloat16),
            "w_k": wk.astype(bfloat16), "w_q": wq.astype(bfloat16),
            "w_v": wv.astype(bfloat16),
            "b_k": bk.astype(np.float32), "b_q": bq.astype(np.float32),
            "b_v": bv.astype(np.float32),
            "w_o": wo.astype(bfloat16), "b_o": bo.astype(np.float32),
            "masks": masks.astype(bfloat16),
        })
    return in_maps


_NC_CACHE = {}


def _get_nc(key, **kw):
    if key not in _NC_CACHE:
        _NC_CACHE[key] = build_nc(**kw)
    return _NC_CACHE[key]


def kernel(x, W_in, b_in, W_out, b_out):
    x = np.asarray(x, dtype=np.float32)
    W_in = np.asarray(W_in, dtype=np.float32)
    b_in = np.asarray(b_in, dtype=np.float32)
    W_out = np.asarray(W_out, dtype=np.float32)
    b_out = np.asarray(b_out, dtype=np.float32)

    B, S, D = x.shape          # 4, 2048, 1024
    HD = 64
    H_pc = (D // HD) // 2      # 8 heads per core

    nc = _get_nc((S, D, H_pc, 1), S=S, D=D, H_pc=H_pc, HD=HD, reps=1)
    in_maps = make_in_maps(x, W_in, b_in, W_out, b_out, S, D, H_pc, HD)
    res = run_bass_kernel_spmd(nc, in_maps, list(range(2 * B)))
    outs = res.results
    out = np.empty((B, S, D), dtype=np.float32)
    for b in range(B):
        out[b] = (outs[2 * b]["out_t"] + outs[2 * b + 1]["out_t"]).T
    return out


def _pjrt_runner(nc, n_cores):
    """Cached jitted 8-core runner with no donation, for steady-state timing."""
    import jax
    from jax.sharding import Mesh, PartitionSpec, NamedSharding
    from jax.experimental.shard_map import shard_map
    from concourse import bass2jax, mybir as mb
    bass2jax.install_neuronx_cc_hook()

    partition_name = nc.partition_id_tensor.name if nc.partition_id_tensor else None
    in_names, out_names, out_avals, zero_outs = [], [], [], []
    for alloc in nc.m.functions[0].allocations:
        if not isinstance(alloc, mb.MemoryLocationSet):
            continue
        name = alloc.memorylocations[0].name
        if alloc.kind == "ExternalInput":
            if name != partition_name:
                in_names.append(name)
        elif alloc.kind == "ExternalOutput":
            out_names.append(name)
            shape = tuple(alloc.tensor_shape)
            dtype = mb.dt.np(alloc.dtype)
            out_avals.append(jax.core.ShapedArray(shape, dtype))
            zero_outs.append(np.zeros(shape, dtype))
    n_params = len(in_names)
    all_names = in_names + out_names
    if partition_name is not None:
        all_names = all_names + [partition_name]

    def _body(*args):
        operands = list(args)
        if partition_name is not None:
            operands.append(bass2jax.partition_id_tensor())
        outs = bass2jax._bass_exec_p.bind(
            *operands,
            out_avals=tuple(out_avals),
            in_names=tuple(all_names),
            out_names=tuple(out_names),
            lowering_input_output_aliases=(),
            sim_require_finite=True,
            sim_require_nnan=True,
            nc=nc,
        )
        return tuple(outs)

    devices = jax.devices()[:n_cores]
    mesh = Mesh(np.asarray(devices), ("core",))
    spec = PartitionSpec("core")
    f = jax.jit(shard_map(
        _body, mesh=mesh,
        in_specs=(spec,) * (n_params + len(out_names)),
        out_specs=(spec,) * len(out_names),
        check_rep=False,
    ))
    sharding = NamedSharding(mesh, spec)
    return f, in_names, zero_outs, sharding, out_names


def _timed_min(f, args, iters):
    import time as _time
    import jax
    out = f(*args)
    jax.block_until_ready(out)  # warmup + compile
    times = []
    for _ in range(iters):
        t0 = _time.perf_counter()
        out = f(*args)
        jax.block_until_ready(out)
        times.append(_time.perf_counter() - t0)
    return min(times)


def _device_args(in_maps, in_names, zero_outs, sharding):
    import jax
    n_cores = len(in_maps)
    args = []
    for name in in_names:
        g = np.concatenate([np.asarray(in_maps[c][name]) for c in range(n_cores)], axis=0)
        args.append(jax.device_put(g, sharding))
    for z in zero_outs:
        g = np.concatenate([z] * n_cores, axis=0)
        args.append(jax.device_put(g, sharding))
    return args


TIMING_REPS = 33


def time_kernel(x, W_in, b_in, W_out, b_out, iters=10):
    """Steady-state HW exec time (ns) of one 8-core kernel execution.

    The axon tunnel adds a large, variable per-dispatch round-trip latency
    (tens of ms) that has nothing to do with kernel speed, so per-call wall
    time cannot resolve the kernel. Instead we compile a second NEFF whose
    body is the identical kernel wrapped in an on-device For_i loop with
    TIMING_REPS iterations, measure blocking wall time of both (min over
    `iters` calls, inputs device-resident), and report the slope
        (T_reps - T_1) / (TIMING_REPS - 1),
    which cancels the dispatch round-trip exactly and yields the marginal
    cost of one kernel execution on the hardware (DMA from HBM included:
    every iteration reloads x/W from HBM and writes the full output).
    """
    x = np.asarray(x, dtype=np.float32)
    B, S, D = x.shape
    HD = 64
    H_pc = (D // HD) // 2
    in_maps = make_in_maps(np.asarray(x), np.asarray(W_in), np.asarray(b_in),
                           np.asarray(W_out), np.asarray(b_out), S, D, H_pc, HD)
    n_cores = len(in_maps)

    nc1 = _get_nc((S, D, H_pc, 1), S=S, D=D, H_pc=H_pc, HD=HD, reps=1)
    ncR = _get_nc((S, D, H_pc, TIMING_REPS), S=S, D=D, H_pc=H_pc, HD=HD,
                  reps=TIMING_REPS)

    f1, in_names, zero_outs, sharding, _ = _pjrt_runner(nc1, n_cores)
    args = _device_args(in_maps, in_names, zero_outs, sharding)
    t1 = _timed_min(f1, args, iters)

    fR, in_namesR, zero_outsR, shardingR, _ = _pjrt_runner(ncR, n_cores)
    argsR = _device_args(in_maps, in_namesR, zero_outsR, shardingR)
    tR = _timed_min(fR, argsR, iters)

    return (tR - t1) / (TIMING_REPS - 1) * 1e9

